# revision 16
# baseline (speedup 1.0000x reference)
"""Croston's method recurrence kernel for Trainium2 (Bass/Tile), 8-core SPMD.

Reference semantics (per series b, scanned over time t):
    nz  = x_t != 0
    Z_t = nz ? a*x_t + (1-a)*Z_{t-1} : Z_{t-1}
    V_t = nz ? a*q_{t-1} + (1-a)*V_{t-1} : V_{t-1}
    q_t = nz ? 1 : q_{t-1} + 1
    out_t = Z_t / V_t

Reformulated exp-free on custom DVE ops (scan() nodes, 1 cyc/elem) instead
of stock affine scans (2.75 cyc/elem). States are scaled by 1/a (cancels in
the ratio) and by scl = gamma^{c0} (chunk centering):

    w_s  = scl * gamma^{-n_s},  n_s = nonzero count within a K-col chunk
    S_Y,t = Yc + sum x_s w_s               -> out_t = S_Y,t / S_W,t
    S_W,t = Wc + sum q_{s-1} nz_s w_s

The q-dependent S_W sum telescopes (Abel): with positions p~ = OFF + s and
a_s = p~ * w_s * nz_s (computable from x and a scaled iota only — no q!),

    S_W,t = C0q + (1-1/g)*cumsum(a)_t + (1/g)*maxscan(a)_t

where one positive "virtual element" a_0 = (OFF+1-Q0)*scl prepended per
chunk carries the boundary state, and C0q = Wc - a_0. Chunk carries for the
next chunk derive from the chunk's S_W,end / sum(a) / nonzero count via a
few [P,1] Pool/Scalar ops. 1/S_W = Exp(-Ln(S_W)); ln/exp/copy/sign share
one activation table set (no table swaps); the host picks (K, c0) so S_W
stays inside Ln's usable input range e^[-43, 43].

Per 128x2048 tile the DVE runs 4 custom passes (u_Y, a, S_W, and
S_Y-cumsum*reciprocal fused) — no q pass at all.
"""

import numpy as np
from contextlib import ExitStack

import concourse.bass as bass
import concourse.mybir as mybir
from concourse import tile
from concourse.bass_utils import run_bass_kernel_spmd
from concourse.library_overlay import lower_extended_insts

B, T = 8192, 2048
N_CORES = 8
B_SHARD = B // N_CORES       # 1024 series per core
P = 128                      # SBUF partitions
N_TILES = B_SHARD // P       # 8 row-tiles per core

_DT = mybir.dt.float32
_OP = mybir.AluOpType
_ACT = mybir.ActivationFunctionType
FLT_MAX = np.float32(3.4028235e38)

TRACE = False                # set by test harness to capture a HW profile
LAST_RESULTS = None          # BassKernelResults of the last run (for test.py)

_nc_cache: dict[tuple, object] = {}

# --------------------------------------------------------------------------
# Custom DVE ops (registered idempotently into concourse.dve_ops)
# --------------------------------------------------------------------------
_ops_registered = False
_ops = {}


def _register_ops():
    global _ops_registered
    if _ops_registered:
        return
    from concourse import dve_ops
    from concourse.dve_ops import (
        DveOp,
        OPS,
        _SUB_OPCODE_FOR_NAME,
        _CUSTOM_DVE_ROW_BASE,
    )
    from concourse.dve_spec import (
        Spec,
        Src0,
        Src1,
        C0,
        C1,
        C2,
        Zero,
        One,
        MaxNeg,
        AluOp,
        ne,
        select,
        scan,
        lower,
    )
    from concourse.dve_uop import DveOpSpec

    def _w_of(x, g, w0=1.0):
        return np.float64(w0) * np.cumprod(
            np.where(x != 0.0, np.float64(g), 1.0), axis=1
        )

    def _ref_uy(in0, in1, s0, s1, imm2):
        Pn = in0.shape[0]
        x = np.asarray(in0, np.float64).reshape(Pn, -1)
        u = (x * _w_of(x, s0, imm2)).astype(np.float32)
        acc = np.asarray(s1, np.float64).reshape(-1, 1) + u.astype(
            np.float64
        ).sum(axis=1, keepdims=True)
        return u, acc.astype(np.float32)

    def _ref_a(in0, in1, s0, s1, imm2):
        Pn = in0.shape[0]
        x = np.asarray(in0, np.float64).reshape(Pn, -1)
        io = np.asarray(in1, np.float64).reshape(Pn, -1)
        u = (io * _w_of(x, s0) * (x != 0.0)).astype(np.float32)
        return u, u.astype(np.float64).sum(axis=1, keepdims=True).astype(
            np.float32
        )

    def _ref_swv2(in0, in1, s0, s1, imm2):
        Pn = in0.shape[0]
        av = np.asarray(in0, np.float64).reshape(Pn, -1)
        r = np.maximum.accumulate(
            np.where(av > 0.0, av, -np.float64(FLT_MAX)), axis=1
        )
        At = np.cumsum(av, axis=1) + np.asarray(s0, np.float64).reshape(-1, 1)
        return (r * np.float64(s1) + At * np.float64(imm2)).astype(np.float32)

    def _ref_sydiv(in0, in1, s0, s1, imm2):
        Pn = in0.shape[0]
        u = np.asarray(in0, np.float64).reshape(Pn, -1)
        r = np.asarray(in1, np.float64).reshape(Pn, -1)
        return (
            (np.asarray(s0, np.float64).reshape(-1, 1) + np.cumsum(u, axis=1))
            * r
        ).astype(np.float32)

    defs = [
        # u_Y pass: Src0 = x; C0 = 1/gamma lit, C1 = Yc [P,1] (accum seed),
        # C2(imm) = scl. out = x*w; accum_out = Yc + sum(out) = S_Y,end
        (
            "CROSTON_UY_ANT",
            Spec(
                body=Src0
                * scan(
                    AluOp.MULTIPLY, select(ne(Src0, Zero), C0, One), init=C2
                ),
                accum=AluOp.ADD,
                accum_init=C1,
                reference=_ref_uy,
            ),
        ),
        # a pass: Src0 = x, Src1 = (OFF+1..OFF+K)*scl (elementwise const);
        # C0 = 1/gamma lit. out = Src1 * cumprod(nz?1/g:1) * nz;
        # accum_out = sum(out)
        (
            "CROSTON_A_ANT",
            Spec(
                body=(
                    Src1
                    * scan(AluOp.MULTIPLY, select(ne(Src0, Zero), C0, One))
                )
                * ne(Src0, Zero),
                accum=AluOp.ADD,
                reference=_ref_a,
            ),
        ),
        # S_W pass over [a0, a_1..a_K]: C0 = C0q/(1-1/g) [P,1],
        # C1 = 1/g lit, C2(imm) = 1-1/g.
        # out = maxscan(a>0 ? a : -inf)*C1 + (cumsum(a)+C0)*C2
        (
            "CROSTON_SWV2_ANT",
            Spec(
                body=scan(AluOp.MAX, select(Src0 > Zero, Src0, MaxNeg)) * C1
                + scan(AluOp.ADD, Src0, init=C0) * C2,
                reference=_ref_swv2,
            ),
        ),
        # final: Src0 = u_Y, Src1 = 1/S_W (elementwise); C0 = Yc [P,1].
        (
            "CROSTON_SYDIV_ANT",
            Spec(
                body=scan(AluOp.ADD, Src0, init=C0) * Src1,
                reference=_ref_sydiv,
            ),
        ),
    ]

    names = {op.name for op in OPS}
    for name, spec in defs:
        sha = {}
        for ver in ("v3", "v4"):
            sha[ver] = DveOpSpec(
                name=name, opcode=0, uops=lower(spec, ver=ver), rd1_en=False
            ).sha(ver)
        op = DveOp(name, spec, subdim=False, uops_sha=sha)
        _ops[name] = op
        if name in names:
            continue
        OPS.append(op)
        _SUB_OPCODE_FOR_NAME[name] = _CUSTOM_DVE_ROW_BASE + len(OPS) - 1
        dve_ops.CUSTOM_DVE_SPECS[name] = spec
    assert max(_SUB_OPCODE_FOR_NAME.values()) < 0x20
    _ops_registered = True


# --------------------------------------------------------------------------
# Program build
# --------------------------------------------------------------------------


def _split_tsp_waits(nc):
    """walrus's codegen accepts at most one embedded sync wait per compute
    instruction (and none on InstCustomDveAnt/InstISA). Hoist excess waits
    onto single-wait NoOps just before the instruction in its engine queue."""
    skip = (mybir.InstNoOp,)
    zero_wait = (mybir.InstCustomDveAnt, mybir.InstISA)
    for fn in nc.m.functions:
        for blk in fn.blocks:
            out = []
            for inst in blk.instructions:
                si = inst.sync_info
                if (
                    not isinstance(inst, skip)
                    and si is not None
                    and len(si.on_wait)
                    > (0 if isinstance(inst, zero_wait) else 1)
                ):
                    for k, w in enumerate(si.on_wait):
                        nop = mybir.InstNoOp(name=f"{inst.name}-w{k}")
                        nop.engine = inst.engine
                        nop.sync_info = mybir.SyncInfo(on_wait=[w], on_update=[])
                        out.append(nop)
                    inst.sync_info = mybir.SyncInfo(
                        on_wait=[], on_update=si.on_update
                    )
                out.append(inst)
            blk.instructions = out


def _build_nc(a: float, K: int, c0: int, OFF: int):
    _register_ops()
    UY_OP = _ops["CROSTON_UY_ANT"]
    A_OP = _ops["CROSTON_A_ANT"]
    SW_OP = _ops["CROSTON_SWV2_ANT"]
    SYDIV_OP = _ops["CROSTON_SYDIV_ANT"]

    NCH = T // K
    KB = K + 1  # chunk block width incl. the virtual column
    gamma = float(np.float32(1.0) - np.float32(a))
    inv_g = float(1.0 / np.float32(gamma))
    ln_g = float(np.log(np.float64(gamma)))
    scl = float(np.float64(gamma) ** c0)
    inv_a_scl = float(np.float64(scl) / np.float64(a))
    one_m_invg = float(1.0 - np.float64(inv_g))
    kap = float(1.0 / one_m_invg)
    g_m1 = float(np.float64(gamma) - 1.0)
    mKscl = float(-np.float64(K) * np.float64(scl))
    a0_scale = float(-np.float64(scl))
    a0_bias = float((np.float64(OFF) + 1.0) * np.float64(scl))

    nc = bass.Bass()
    x = nc.dram_tensor("x", [B_SHARD, T], _DT, kind="ExternalInput")
    iota = nc.dram_tensor("iota", [P, K], _DT, kind="ExternalInput")
    z0 = nc.dram_tensor("z0", [P, N_TILES], _DT, kind="ExternalInput")
    v0 = nc.dram_tensor("v0", [P, N_TILES], _DT, kind="ExternalInput")
    q0 = nc.dram_tensor("q0", [P, N_TILES], _DT, kind="ExternalInput")
    out = nc.dram_tensor("out", [B_SHARD, T], _DT, kind="ExternalOutput")

    xv = x[:].rearrange("(n p) t -> n p t", p=P)
    ov = out[:].rearrange("(n p) t -> n p t", p=P)

    with tile.TileContext(nc) as tc:
        with ExitStack() as ctx:
            const = ctx.enter_context(tc.tile_pool(name="const", bufs=1))
            xp0 = ctx.enter_context(tc.tile_pool(name="xp0", bufs=1))
            iotat = const.tile([P, K], _DT, tag="iota")
            nc.sync.dma_start(iotat[:], iota[:])
            xt_first = xp0.tile([P, T], _DT, tag="x0")
            for c in range(T // K):
                nc.sync.dma_start(
                    xt_first[:, c * K : (c + 1) * K],
                    xv[0][:, c * K : (c + 1) * K],
                )
            q0s = const.tile([P, N_TILES], _DT, tag="q0s")
            z0s = const.tile([P, N_TILES], _DT, tag="z0s")
            v0s = const.tile([P, N_TILES], _DT, tag="v0s")
            nc.sync.dma_start(z0s[:], z0[:])
            nc.sync.dma_start(v0s[:], v0[:])
            nc.sync.dma_start(q0s[:], q0[:])
            # tile-start carries in the scl frame: Yc0/Wc0 = (Z0|V0)*scl/a
            ones1 = const.tile([P, 1], _DT, tag="ones1")
            nc.gpsimd.memset(ones1[:], 1.0)
            yc0 = const.tile([P, N_TILES], _DT, tag="yc0")
            wc0 = const.tile([P, N_TILES], _DT, tag="wc0")
            a00 = const.tile([P, N_TILES], _DT, tag="a00")
            nc.scalar.activation(yc0[:], z0s[:], _ACT.Copy, scale=inv_a_scl)
            nc.scalar.activation(wc0[:], v0s[:], _ACT.Copy, scale=inv_a_scl)
            # tile-start virtual element (OFF+1-q0)*scl
            nc.scalar.activation(
                a00[:], q0s[:], _ACT.Copy, scale=a0_scale, bias=a0_bias
            )

            xp = ctx.enter_context(tc.tile_pool(name="xp", bufs=3))
            wp = ctx.enter_context(tc.tile_pool(name="wp", bufs=3))
            op_ = ctx.enter_context(tc.tile_pool(name="op", bufs=3))

            pend = None  # deferred back-half of the previous tile

            def emit_back(p):
                # per-chunk ln/exp reciprocal + final SYDIV + store
                (uy, avt, swt, ycols, i) = p
                lnv, rv = avt, swt  # reuse dead tiles
                ot = op_.tile([P, T], _DT, tag="o")
                ovi = ov[i]
                for c in range(NCH):
                    bs = slice(c * KB, (c + 1) * KB)
                    rs = slice(c * KB + 1, (c + 1) * KB)
                    sl = slice(c * K, (c + 1) * K)
                    nc.scalar.activation(lnv[:, bs], swt[:, bs], _ACT.Ln)
                    nc.scalar.activation(
                        rv[:, bs], lnv[:, bs], _ACT.Exp, scale=-1.0
                    )
                    nc.vector._custom_dve(
                        SYDIV_OP,
                        out=ot[:, sl],
                        in0=uy[:, sl],
                        in1=rv[:, rs].rearrange("p (o n) -> p o n", o=1),
                        s0=ycols[c],
                    )
                    nc.sync.dma_start(ovi[:, sl], ot[:, sl])

            for i in range(N_TILES):
                if i == 0:
                    xt = xt_first
                else:
                    xt = xp.tile([P, T], _DT, tag="x")
                    for c in range(NCH):
                        nc.sync.dma_start(
                            xt[:, c * K : (c + 1) * K],
                            xv[i][:, c * K : (c + 1) * K],
                        )

                uy = wp.tile([P, T], _DT, tag="uy")
                avt = wp.tile([P, NCH * KB], _DT, tag="avt")
                swt = wp.tile([P, NCH * KB], _DT, tag="swt")
                mscr = wp.tile([P, K], _DT, tag="mscr")
                cnt = wp.tile([P, NCH], _DT, tag="cnt")
                bco = wp.tile([P, NCH], _DT, tag="bco")
                ycc = wp.tile([P, NCH], _DT, tag="ycc")
                wcc = wp.tile([P, NCH], _DT, tag="wcc")
                aend = wp.tile([P, NCH], _DT, tag="aend")
                c0q = wp.tile([P, NCH], _DT, tag="c0q")
                c0sl = wp.tile([P, NCH], _DT, tag="c0sl")
                scr = wp.tile([P, 4], _DT, tag="scr")

                # per-chunk nonzero counts (Scalar Sign + accumulate; Sign
                # shares the natural_log_exp act table set -> no swaps)
                for c in range(NCH):
                    sl = slice(c * K, (c + 1) * K)
                    nc.scalar.activation(
                        mscr[:],
                        xt[:, sl],
                        _ACT.Sign,
                        accum_out=cnt[:, c : c + 1],
                    )
                nc.scalar.activation(bco[:], cnt[:], _ACT.Exp, scale=ln_g)

                def ycol(c, i=i, ycc=ycc):
                    return yc0[:, i : i + 1] if c == 0 else ycc[:, c : c + 1]

                def a0col(c, avt=avt):
                    return avt[:, c * KB : c * KB + 1]

                # chunk-0 virtual element + C0 slot (Pool copy via *1)
                nc.gpsimd.tensor_tensor(
                    avt[:, 0:1], a00[:, i : i + 1], ones1[:], _OP.mult
                )
                nc.gpsimd.tensor_tensor(
                    c0q[:, 0:1], wc0[:, i : i + 1], avt[:, 0:1], _OP.subtract
                )
                nc.scalar.activation(
                    c0sl[:, 0:1], c0q[:, 0:1], _ACT.Copy, scale=kap
                )

                # a passes first (independent of all carries)
                for c in range(NCH):
                    sl = slice(c * K, (c + 1) * K)
                    nc.vector._custom_dve(
                        A_OP,
                        out=avt[:, c * KB + 1 : (c + 1) * KB],
                        in0=xt[:, sl],
                        in1=iotat[:].rearrange("p (o n) -> p o n", o=1),
                        s0=inv_g,
                        accum_out=aend[:, c : c + 1],
                    )

                for c in range(NCH):
                    if pend is not None and c == NCH - 1:
                        emit_back(pend)
                        pend = None
                    sl = slice(c * K, (c + 1) * K)
                    last = c + 1 >= NCH
                    nc.vector._custom_dve(
                        UY_OP,
                        out=uy[:, sl],
                        in0=xt[:, sl],
                        s0=inv_g,
                        s1=ycol(c),
                        imm2=scl,
                        accum_out=(None if last else ycc[:, c + 1 : c + 2]),
                    )
                    if not last:
                        nc.gpsimd.tensor_tensor(
                            ycc[:, c + 1 : c + 2],
                            ycc[:, c + 1 : c + 2],
                            bco[:, c : c + 1],
                            _OP.mult,
                        )
                    nc.vector._custom_dve(
                        SW_OP,
                        out=swt[:, c * KB : (c + 1) * KB],
                        in0=avt[:, c * KB : (c + 1) * KB],
                        s0=c0sl[:, c : c + 1],
                        s1=inv_g,
                        imm2=one_m_invg,
                    )
                    if not last:
                        # boundary: derive next chunk's virtual elem + carries
                        swend = swt[:, (c + 1) * KB - 1 : (c + 1) * KB]
                        # u2 = gamma * (S_W,end - C0q)
                        nc.gpsimd.tensor_tensor(
                            scr[:, 0:1], swend, c0q[:, c : c + 1], _OP.subtract
                        )
                        nc.scalar.activation(
                            scr[:, 0:1], scr[:, 0:1], _ACT.Copy, scale=gamma
                        )
                        # v = (gamma-1) * (a0 + sum a)
                        nc.gpsimd.tensor_tensor(
                            scr[:, 1:2],
                            a0col(c),
                            aend[:, c : c + 1],
                            _OP.add,
                        )
                        nc.scalar.activation(
                            scr[:, 1:2], scr[:, 1:2], _ACT.Copy, scale=g_m1
                        )
                        # rte = u2 - v ;  m = rte * B_c
                        nc.gpsimd.tensor_tensor(
                            scr[:, 2:3], scr[:, 0:1], scr[:, 1:2], _OP.subtract
                        )
                        nc.gpsimd.tensor_tensor(
                            scr[:, 2:3],
                            scr[:, 2:3],
                            bco[:, c : c + 1],
                            _OP.mult,
                        )
                        # a0_next = m - K*scl  (virtual col of next chunk)
                        nc.scalar.activation(
                            avt[:, (c + 1) * KB : (c + 1) * KB + 1],
                            scr[:, 2:3],
                            _ACT.Copy,
                            bias=mKscl,
                        )
                        # Wc_next = B_c * S_W,end ; next C0q and C0 slot
                        nc.gpsimd.tensor_tensor(
                            wcc[:, c + 1 : c + 2],
                            swend,
                            bco[:, c : c + 1],
                            _OP.mult,
                        )
                        nc.gpsimd.tensor_tensor(
                            c0q[:, c + 1 : c + 2],
                            wcc[:, c + 1 : c + 2],
                            avt[:, (c + 1) * KB : (c + 1) * KB + 1],
                            _OP.subtract,
                        )
                        nc.scalar.activation(
                            c0sl[:, c + 1 : c + 2],
                            c0q[:, c + 1 : c + 2],
                            _ACT.Copy,
                            scale=kap,
                        )

                if pend is not None:
                    emit_back(pend)
                pend = (uy, avt, swt, [ycol(c) for c in range(NCH)], i)

            emit_back(pend)
    _split_tsp_waits(nc)
    lower_extended_insts(nc)
    return nc


def _pick_K(a: float, x: np.ndarray, Z0, V0, q0):
    """Pick (K, c0): the largest power-of-2 chunk K (<=1024) and centering
    exponent c0 (scl = gamma^c0) such that, for THIS input, S_W stays inside
    the Scalar Ln's usable range e^[-43, 43], intermediates stay fp32-normal
    (|ln| < 80), and gamma^{n_chunk} stays normal."""
    gamma = float(np.float64(1.0) - np.float64(np.float32(a)))
    if gamma <= 0.0 or gamma >= 1.0 - 1e-9:
        return 1024, 0
    eta = -np.log(gamma)  # > 0

    nz = x != 0.0
    czs = np.cumsum(~nz, axis=1, dtype=np.int64)
    run = czs - np.maximum.accumulate(np.where(nz, czs, 0), axis=1)
    qmax = float(run.max()) + float(np.abs(q0).max()) + 2.0
    aa = max(float(np.float32(a)), 1e-12)
    wmax0 = float(np.abs(V0).max()) / aa + 1.0
    wmin0 = max(min(float(np.abs(V0).min()) / aa, 1e6), 1e-6)
    sum_hi = np.log(qmax / max(1.0 - gamma, 1e-6) + wmax0 + 2.0)

    for K in (1024, 512, 256, 128, 64, 32, 16, 8):
        if T % K:
            continue
        cmax = int(
            nz.reshape(x.shape[0], T // K, K).sum(axis=2, dtype=np.int64).max()
        )
        if cmax * eta > 85.0:  # gamma^{n_c} carry factor would denormal
            continue
        # the a/A internals carry an extra ~(2T+16) position factor vs S_W
        pad = np.log(2.0 * T + 16.0)
        c0_lo = (cmax * eta + sum_hi + pad - 80.0) / eta
        c0_lo = max(c0_lo, (cmax * eta + sum_hi - 43.0) / eta)
        c0_hi = (43.0 + np.log(wmin0)) / eta
        c0_lo = max(c0_lo, K - (80.0 - np.log(qmax)) / eta)
        c0_hi = min(c0_hi, 80.0 / eta)
        if c0_lo <= c0_hi:
            c0 = int(round((max(c0_lo, 0.0) + c0_hi) / 2.0))
            return K, c0
    return 8, 0


def _get_nc(a: float, K: int, c0: int, OFF: int):
    key = (int(np.float32(a).view(np.int32)), K, c0, OFF)
    nc = _nc_cache.get(key)
    if nc is None:
        nc = _build_nc(a, K, c0, OFF)
        _nc_cache[key] = nc
    return nc


def kernel(x, alpha, Z0, V0, q0):
    global LAST_RESULTS
    x = np.ascontiguousarray(np.asarray(x, dtype=np.float32))
    a = float(np.asarray(alpha, dtype=np.float32).reshape(-1)[0])
    Z0 = np.asarray(Z0, dtype=np.float32).reshape(B, 1)
    V0 = np.asarray(V0, dtype=np.float32).reshape(B, 1)
    q0 = np.asarray(q0, dtype=np.float32).reshape(B, 1)

    if not (0.0 < a < 1.0) or (x < 0).any() or (q0 < 0).any():
        # degenerate smoothing weight / negative demands: exact CPU path
        return _cpu_reference(x, a, Z0, V0, q0)

    K, c0 = _pick_K(a, x, Z0, V0, q0)
    OFF = T + int(np.ceil(float(q0.max()))) + 8
    nc = _get_nc(a, K, c0, OFF)

    scl = np.float64(1.0 - np.float32(a)) ** c0
    iota = ((np.arange(K, dtype=np.float64) + 1.0 + OFF) * scl).astype(
        np.float32
    )
    iota = np.ascontiguousarray(np.broadcast_to(iota, (P, K)))

    def _cols(v, k):
        # [P, N_TILES] layout: column i holds series (k*B_SHARD + i*128 + p)
        sh = v[k * B_SHARD : (k + 1) * B_SHARD, 0].reshape(N_TILES, P)
        return np.ascontiguousarray(sh.T)

    in_maps = []
    for k in range(N_CORES):
        s = slice(k * B_SHARD, (k + 1) * B_SHARD)
        in_maps.append(
            {
                "x": x[s],
                "iota": iota,
                "z0": _cols(Z0, k),
                "v0": _cols(V0, k),
                "q0": _cols(q0, k),
            }
        )

    res = run_bass_kernel_spmd(nc, in_maps, list(range(N_CORES)), trace=TRACE)
    LAST_RESULTS = res
    return np.concatenate(
        [res.results[k]["out"] for k in range(N_CORES)], axis=0
    )


def _cpu_reference(x, a, Z0, V0, q0):
    Z = Z0[:, 0].astype(np.float64).copy()
    V = V0[:, 0].astype(np.float64).copy()
    q = q0[:, 0].astype(np.float64).copy()
    outs = np.empty_like(x)
    for t in range(T):
        xt = x[:, t].astype(np.float64)
        nz = xt != 0
        Z = np.where(nz, a * xt + (1 - a) * Z, Z)
        V = np.where(nz, a * q + (1 - a) * V, V)
        q = np.where(nz, 1.0, q + 1.0)
        outs[:, t] = (Z / V).astype(np.float32)
    return outs
